# revision 17
# baseline (speedup 1.0000x reference)
"""Trainium2 Bass kernel for nn_NeuralNetwork_42975442764251.

Reference computes:
    A = I + a0*shift(844) + a1*shift(1264)   (8137x8137, unit upper-triangular)
    loss = sum( (q @ inv(A))[0, 0, -1264:] ** 2 )

Since N = A - I is nilpotent (N^10 = 0), inv(A) = sum_{k=0..9} (-1)^k N^k
exactly: a sparse FIR on q[0,0,:] with taps at s = 844*k + 420*j
(0 <= j <= k <= 9, s <= 8136) and coefficients
    c_{k,j} = (-1)^k * C(k,j) * a0^(k-j) * a1^j.

Only x[0,0,m] for m in [6873, 8137) matters.  Sharding: the 1264 output
positions are split 8 ways (158 per core).  The tap set is embedded in the
full 10x10 (k,j) lattice (100 partitions, invalid taps get coefficient 0),
whose window bases are linear in (k,j) — so all windows load with ONE 3-D
strided DMA.  One K=100 matmul contracts taps; a single fused ACT
Square+accumulate produces the per-core partial; host sums 8 partials.

Raw Bass (no Tile): explicit semaphores keep every instruction at <=1
embedded sync wait (this walrus build rejects more), and the tail is just
the short fixed Bass end-of-block drain (no Tile sem-reset barriers).  A
dummy ACT op at stream start prefetches the activation table set (~2.7us)
in parallel with the input DMAs.

Coefficient modes (env BASS_COEF_MODE, default "dev"):
  dev:  coefficients computed on-device from a0/a1 via Ln/Exp on ACT
  host: coefficients precomputed on host, passed as the param tensor
"""

import os
import numpy as np
from math import comb

T0, T1, L = 844, 1264, 8137
M0 = L - T1            # 6873: first output position
NCORES = 8
NPOS = T1 // NCORES    # 158 output positions per core
NK = 10                # k lattice size
NT = NK * NK           # 100 lattice slots (40 valid taps)
PAD = 844 * 9 + 420 * 9 - M0   # 4503: zero left-pad so every base >= 0
QLEN = 844 * 9 + 420 * 9 + NPOS  # 11534 per-core input length

COEF_MODE = os.environ.get("BASS_COEF_MODE", "dev")


def _coef_table():
    """[100, 3] per-lattice-slot table: [e0, e1, b].

    Slot p = a*10 + bb maps to k = 9-a, j = 9-bb (so window bases
    844*a + 420*bb are nonnegative).  b = 0 for invalid taps.
    """
    tbl = np.zeros((NT, 3), dtype=np.float32)
    for a in range(NK):
        for bb in range(NK):
            k, j = 9 - a, 9 - bb
            p = a * NK + bb
            valid = (j <= k) and (844 * k + 420 * j <= L - 1)
            tbl[p, 0] = k - j
            tbl[p, 1] = j
            tbl[p, 2] = float((-1) ** k * comb(k, j)) if valid else 0.0
    return tbl


def _host_coefs(a0, a1):
    tbl = _coef_table()
    c = np.zeros((NT, 1), dtype=np.float32)
    for p in range(NT):
        e0, e1, b = tbl[p]
        if b != 0.0:
            c[p, 0] = np.float32(b) * np.float32(a0) ** np.float32(e0) * np.float32(a1) ** np.float32(e1)
    return c


_NC_CACHE = {}


def _build_nc(mode):
    import concourse.bass as bass
    import concourse.mybir as mybir

    f32 = mybir.dt.float32
    AF = mybir.ActivationFunctionType
    nc = bass.Bass()
    qc = nc.declare_dram_parameter("qc", [QLEN], f32, isOutput=False)
    ncols = 5 if mode == "dev" else 1
    pp = nc.declare_dram_parameter("pp", [NT, ncols], f32, isOutput=False)
    out = nc.declare_dram_parameter("out", [1, 1], f32, isOutput=True)

    with (
        nc.sbuf_tensor([NT, NPOS], f32) as W,
        nc.sbuf_tensor([NT, ncols], f32) as P,
        nc.sbuf_tensor([NT, 2], f32) as lnA,
        nc.sbuf_tensor([NT, 1], f32) as b1,
        nc.sbuf_tensor([NT, 1], f32) as pw,
        nc.sbuf_tensor([NT, 1], f32) as coef,
        nc.psum_tensor([1, NPOS], f32) as x,
        nc.sbuf_tensor([1, NPOS], f32) as sq,
        nc.sbuf_tensor([1, 1], f32) as warm,
        nc.sbuf_tensor([1, 1], f32) as loss,
        nc.semaphore("dsem") as dsem,
        nc.semaphore("asem") as asem,
        nc.semaphore("psem") as psem,
        nc.semaphore("osem") as osem,
        nc.Block() as block,
    ):
        a_done = 5 if mode == "dev" else 1  # asem value when loss is ready

        @block.sync
        def _(sync):
            src = bass.AP(qc[:].tensor, 0, [[844, NK], [420, NK], [1, NPOS]])
            sync.dma_start(W[:], src).then_inc(dsem, 16)
            sync.dma_start(P[:], pp[:]).then_inc(dsem, 16)
            sync.wait_ge(asem, a_done)
            sync.dma_start(out[:], loss[:], single_packet=True).then_inc(osem, 16)
            sync.wait_ge(osem, 16)

        @block.scalar
        def _(scalar):
            # Prefetch the activation table set (~2.7us) in parallel with
            # the input DMAs.  Ln anchors natural_log_exp_and_others, which
            # also contains Exp, Copy and Square; warm's input is the
            # preamble-initialized const-1.0 SBUF, its output is never read.
            one = nc.const_aps.aps[(f32, 1.0)][0:1, 0:1]
            scalar.activation(warm[:], one, AF.Ln if mode == "dev" else AF.Square)
            if mode == "dev":
                # coef[p] = b_p * exp(e0_p*ln(a0) + e1_p*ln(a1))
                # ACT is pipelined: same-engine RAW hazards need semaphore
                # self-syncs (asem doubles as the flush marker).
                scalar.wait_ge(dsem, 32)
                scalar.activation(lnA[:], P[:, 0:2], AF.Ln).then_inc(asem, 1)
                scalar.wait_ge(asem, 1)
                scalar.mul(b1[:], lnA[:, 1:2], P[:, 3:4]).then_inc(asem, 1)
                scalar.wait_ge(asem, 2)
                scalar.activation(
                    pw[:], P[:, 2:3], AF.Exp, bias=b1[:], scale=lnA[:, 0:1]
                ).then_inc(asem, 1)
                scalar.wait_ge(asem, 3)
                scalar.mul(coef[:], pw[:], P[:, 4:5]).then_inc(asem, 1)
            # loss_partial = sum_i x[0, i]^2
            scalar.wait_ge(psem, 1)
            scalar.activation(
                sq[:], x[:], AF.Square, accum_out=loss[:]
            ).then_inc(asem, 1)

        @block.tensor
        def _(tensor):
            tensor.wait_ge(dsem, 32)
            if mode == "dev":
                tensor.wait_ge(asem, 4)
                lhsT = coef[:]
            else:
                lhsT = P[:]
            # x[0, i] = sum_p coef[p] * W[p, i]
            tensor.matmul(x[:], lhsT, W[:], start=True, stop=True).then_inc(psem, 1)

    return nc


def _get_nc(mode=None):
    mode = mode or COEF_MODE
    if mode not in _NC_CACHE:
        _NC_CACHE[mode] = _build_nc(mode)
    return _NC_CACHE[mode]


def _make_in_maps(q, alpha0, alpha1, mode=None):
    mode = mode or COEF_MODE
    q = np.asarray(q, dtype=np.float32)
    a0 = np.float32(alpha0)
    a1 = np.float32(alpha1)
    qpad = np.concatenate([np.zeros(PAD, np.float32), q[0, 0, :]])
    if mode == "dev":
        tbl = _coef_table()
        pp = np.concatenate(
            [np.full((NT, 1), a0, np.float32), np.full((NT, 1), a1, np.float32), tbl],
            axis=1,
        )
    else:
        pp = _host_coefs(a0, a1)
    return [
        {
            "qc": np.ascontiguousarray(qpad[NPOS * c : NPOS * c + QLEN]),
            "pp": pp,
        }
        for c in range(NCORES)
    ]


def _run(q, alpha0, alpha1, mode=None, **spmd_kwargs):
    from concourse.bass_utils import run_bass_kernel_spmd

    mode = mode or COEF_MODE
    in_maps = _make_in_maps(q, alpha0, alpha1, mode)
    nc = _get_nc(mode)
    res = run_bass_kernel_spmd(nc, in_maps, core_ids=list(range(NCORES)), **spmd_kwargs)
    total = np.float32(0.0)
    for r in res.results:
        total = np.float32(total + r["out"][0, 0])
    return np.array(total, dtype=np.float32), res


def kernel(q, alpha0, alpha1):
    out, _ = _run(q, alpha0, alpha1)
    return out


# revision 19
# speedup vs baseline: 1.0958x; 1.0958x over previous
"""Trainium2 Bass kernel for nn_NeuralNetwork_42975442764251.

Reference computes:
    A = I + a0*shift(844) + a1*shift(1264)   (8137x8137, unit upper-triangular)
    loss = sum( (q @ inv(A))[0, 0, -1264:] ** 2 )

Since N = A - I is nilpotent (N^10 = 0), inv(A) = sum_{k=0..9} (-1)^k N^k
exactly: a sparse FIR on q[0,0,:] with taps at s = 844*k + 420*j
(0 <= j <= k <= 9, s <= 8136) and coefficients
    c_{k,j} = (-1)^k * C(k,j) * a0^(k-j) * a1^j.

Only x[0,0,m] for m in [6873, 8137) matters.  Sharding: the 1264 output
positions are split 8 ways (158 per core).  The tap set is embedded in the
full 10x10 (k,j) lattice (100 partitions, invalid taps get coefficient 0),
whose window bases are linear in (k,j) — so all windows load with ONE 3-D
strided DMA.  One K=100 matmul contracts taps; a single fused ACT
Square+accumulate produces the per-core partial; host sums 8 partials.

Raw Bass (no Tile): explicit semaphores keep every instruction at <=1
embedded sync wait (this walrus build rejects more), and the tail is just
the short fixed Bass end-of-block drain (no Tile sem-reset barriers).  A
dummy ACT op at stream start prefetches the activation table set (~2.7us)
in parallel with the input DMAs.

Coefficient modes (env BASS_COEF_MODE, default "dev"):
  dev:  coefficients computed on-device from a0/a1 via Ln/Exp on ACT
  host: coefficients precomputed on host, passed as the param tensor
"""

import os
import numpy as np
from math import comb

T0, T1, L = 844, 1264, 8137
M0 = L - T1            # 6873: first output position
NCORES = 8
NPOS = T1 // NCORES    # 158 output positions per core
NK = 10                # k lattice size
NT = NK * NK           # 100 lattice slots (40 valid taps)
PAD = 844 * 9 + 420 * 9 - M0   # 4503: zero left-pad so every base >= 0
QLEN = 844 * 9 + 420 * 9 + NPOS  # 11534 per-core input length

COEF_MODE = os.environ.get("BASS_COEF_MODE", "dev")
# none: keep bass-emitted preamble/tail barriers; tail: strip the trailing
# per-engine drain+barrier block; all: also strip the preamble barrier
STRIP_MODE = os.environ.get("BASS_STRIP", "none")


def _coef_table():
    """[100, 3] per-lattice-slot table: [e0, e1, b].

    Slot p = a*10 + bb maps to k = 9-a, j = 9-bb (so window bases
    844*a + 420*bb are nonnegative).  b = 0 for invalid taps.
    """
    tbl = np.zeros((NT, 3), dtype=np.float32)
    for a in range(NK):
        for bb in range(NK):
            k, j = 9 - a, 9 - bb
            p = a * NK + bb
            valid = (j <= k) and (844 * k + 420 * j <= L - 1)
            tbl[p, 0] = k - j
            tbl[p, 1] = j
            tbl[p, 2] = float((-1) ** k * comb(k, j)) if valid else 0.0
    return tbl


def _host_coefs(a0, a1):
    tbl = _coef_table()
    c = np.zeros((NT, 1), dtype=np.float32)
    for p in range(NT):
        e0, e1, b = tbl[p]
        if b != 0.0:
            c[p, 0] = np.float32(b) * np.float32(a0) ** np.float32(e0) * np.float32(a1) ** np.float32(e1)
    return c


_NC_CACHE = {}


def _build_nc(mode):
    import concourse.bass as bass
    import concourse.mybir as mybir

    f32 = mybir.dt.float32
    AF = mybir.ActivationFunctionType
    nc = bass.Bass()
    qc = nc.declare_dram_parameter("qc", [QLEN], f32, isOutput=False)
    ncols = 5 if mode == "dev" else 1
    pp = nc.declare_dram_parameter("pp", [NT, ncols], f32, isOutput=False)
    out = nc.declare_dram_parameter("out", [1, 1], f32, isOutput=True)

    with (
        nc.sbuf_tensor([NT, NPOS], f32) as W,
        nc.sbuf_tensor([NT, ncols], f32) as P,
        nc.sbuf_tensor([NT, 2], f32) as lnA,
        nc.sbuf_tensor([NT, 1], f32) as b1,
        nc.sbuf_tensor([NT, 1], f32) as pw,
        nc.sbuf_tensor([NT, 1], f32) as coef,
        nc.psum_tensor([1, NPOS], f32) as x,
        nc.sbuf_tensor([1, NPOS], f32) as sq,
        nc.sbuf_tensor([1, 1], f32) as warm,
        nc.sbuf_tensor([1, 1], f32) as loss,
        nc.semaphore("dsem") as dsem,
        nc.semaphore("asem") as asem,
        nc.semaphore("psem") as psem,
        nc.semaphore("osem") as osem,
        nc.Block() as block,
    ):
        a_done = 5 if mode == "dev" else 1  # asem value when loss is ready

        @block.sync
        def _(sync):
            src = bass.AP(qc[:].tensor, 0, [[844, NK], [420, NK], [1, NPOS]])
            sync.dma_start(W[:], src).then_inc(dsem, 16)
            sync.dma_start(P[:], pp[:]).then_inc(dsem, 16)
            sync.wait_ge(asem, a_done)
            sync.dma_start(out[:], loss[:], single_packet=True).then_inc(osem, 16)
            sync.wait_ge(osem, 16)

        @block.scalar
        def _(scalar):
            # Prefetch the activation table set (~2.7us) in parallel with
            # the input DMAs.  Ln anchors natural_log_exp_and_others, which
            # also contains Exp, Copy and Square; warm's input is the
            # preamble-initialized const-1.0 SBUF, its output is never read.
            one = nc.const_aps.aps[(f32, 1.0)][0:1, 0:1]
            scalar.activation(warm[:], one, AF.Ln if mode == "dev" else AF.Square)
            if mode == "dev":
                # coef[p] = b_p * exp(e0_p*ln(a0) + e1_p*ln(a1))
                # ACT is pipelined: same-engine RAW hazards need semaphore
                # self-syncs (asem doubles as the flush marker).
                scalar.wait_ge(dsem, 32)
                scalar.activation(lnA[:], P[:, 0:2], AF.Ln).then_inc(asem, 1)
                scalar.wait_ge(asem, 1)
                scalar.mul(b1[:], lnA[:, 1:2], P[:, 3:4]).then_inc(asem, 1)
                scalar.wait_ge(asem, 2)
                scalar.activation(
                    pw[:], P[:, 2:3], AF.Exp, bias=b1[:], scale=lnA[:, 0:1]
                ).then_inc(asem, 1)
                scalar.wait_ge(asem, 3)
                scalar.mul(coef[:], pw[:], P[:, 4:5]).then_inc(asem, 1)
            # loss_partial = sum_i x[0, i]^2
            scalar.wait_ge(psem, 1)
            scalar.activation(
                sq[:], x[:], AF.Square, accum_out=loss[:]
            ).then_inc(asem, 1)

        @block.tensor
        def _(tensor):
            tensor.wait_ge(dsem, 32)
            if mode == "dev":
                tensor.wait_ge(asem, 4)
                lhsT = coef[:]
            else:
                lhsT = P[:]
            # x[0, i] = sum_p coef[p] * W[p, i]
            tensor.matmul(x[:], lhsT, W[:], start=True, stop=True).then_inc(psem, 1)

    if STRIP_MODE in ("tail", "all"):
        f = nc.m.functions[0]
        last = f.blocks[-1]
        if all(
            type(i).__name__ in ("InstDrain", "InstEventSemaphore")
            for i in last.instructions
        ):
            # our dataflow is fully semaphore-ordered and the out-DMA is
            # completion-waited via osem; the generic end-of-block engine
            # drains add only span time
            last.instructions = []
    if STRIP_MODE == "all":
        b0 = nc.m.functions[0].blocks[0]
        b0.instructions = [
            i
            for i in b0.instructions
            if type(i).__name__ != "InstDrain" and not i.name.startswith("barrier_")
        ]
    return nc


def _get_nc(mode=None):
    mode = mode or COEF_MODE
    if mode not in _NC_CACHE:
        _NC_CACHE[mode] = _build_nc(mode)
    return _NC_CACHE[mode]


def _make_in_maps(q, alpha0, alpha1, mode=None):
    mode = mode or COEF_MODE
    q = np.asarray(q, dtype=np.float32)
    a0 = np.float32(alpha0)
    a1 = np.float32(alpha1)
    qpad = np.concatenate([np.zeros(PAD, np.float32), q[0, 0, :]])
    if mode == "dev":
        tbl = _coef_table()
        pp = np.concatenate(
            [np.full((NT, 1), a0, np.float32), np.full((NT, 1), a1, np.float32), tbl],
            axis=1,
        )
    else:
        pp = _host_coefs(a0, a1)
    return [
        {
            "qc": np.ascontiguousarray(qpad[NPOS * c : NPOS * c + QLEN]),
            "pp": pp,
        }
        for c in range(NCORES)
    ]


def _run(q, alpha0, alpha1, mode=None, **spmd_kwargs):
    from concourse.bass_utils import run_bass_kernel_spmd

    mode = mode or COEF_MODE
    in_maps = _make_in_maps(q, alpha0, alpha1, mode)
    nc = _get_nc(mode)
    res = run_bass_kernel_spmd(nc, in_maps, core_ids=list(range(NCORES)), **spmd_kwargs)
    total = np.float32(0.0)
    for r in res.results:
        total = np.float32(total + r["out"][0, 0])
    return np.array(total, dtype=np.float32), res


def kernel(q, alpha0, alpha1):
    out, _ = _run(q, alpha0, alpha1)
    return out


# revision 22
# speedup vs baseline: 1.3186x; 1.2033x over previous
"""Trainium2 Bass kernel for nn_NeuralNetwork_42975442764251.

Reference computes:
    A = I + a0*shift(844) + a1*shift(1264)   (8137x8137, unit upper-triangular)
    loss = sum( (q @ inv(A))[0, 0, -1264:] ** 2 )

Since N = A - I is nilpotent (N^10 = 0), inv(A) = sum_{k=0..9} (-1)^k N^k
exactly: a sparse FIR on q[0,0,:] with taps at s = 844*k + 420*j
(0 <= j <= k <= 9, s <= 8136) and coefficients
    c_{k,j} = (-1)^k * C(k,j) * a0^(k-j) * a1^j.

Only x[0,0,m] for m in [6873, 8137) matters.  Sharding: the 1264 output
positions are split 8 ways (158 per core).  The tap set is embedded in the
full 10x10 (k,j) lattice (100 partitions, invalid taps get coefficient 0),
whose window bases are linear in (k,j) — so all windows load with ONE 3-D
strided DMA.  One K=100 matmul contracts taps; a single fused ACT
Square+accumulate produces the per-core partial; host sums 8 partials.

Raw Bass (no Tile): explicit semaphores keep every instruction at <=1
embedded sync wait (this walrus build rejects more), and the tail is just
the short fixed Bass end-of-block drain (no Tile sem-reset barriers).  A
dummy ACT op at stream start prefetches the activation table set (~2.7us)
in parallel with the input DMAs.

Coefficient modes (env BASS_COEF_MODE, default "dev"):
  dev:  coefficients computed on-device from a0/a1 via Ln/Exp on ACT
  host: coefficients precomputed on host, passed as the param tensor
"""

import os
import numpy as np
from math import comb

T0, T1, L = 844, 1264, 8137
M0 = L - T1            # 6873: first output position
NCORES = 8
NPOS = T1 // NCORES    # 158 output positions per core
NK = 10                # k lattice size
NT = NK * NK           # 100 lattice slots (40 valid taps)
PAD = 844 * 9 + 420 * 9 - M0   # 4503: zero left-pad so every base >= 0
QLEN = 844 * 9 + 420 * 9 + NPOS  # 11534 per-core input length

COEF_MODE = os.environ.get("BASS_COEF_MODE", "dev")
# none: keep bass-emitted preamble/tail barriers; tail: strip the trailing
# per-engine drain+barrier block; all: also strip the preamble barrier
STRIP_MODE = os.environ.get("BASS_STRIP", "none")


def _coef_table():
    """[100, 3] per-lattice-slot table: [e0, e1, b].

    Slot p = a*10 + bb maps to k = 9-a, j = 9-bb (so window bases
    844*a + 420*bb are nonnegative).  b = 0 for invalid taps.
    """
    tbl = np.zeros((NT, 3), dtype=np.float32)
    for a in range(NK):
        for bb in range(NK):
            k, j = 9 - a, 9 - bb
            p = a * NK + bb
            valid = (j <= k) and (844 * k + 420 * j <= L - 1)
            tbl[p, 0] = k - j
            tbl[p, 1] = j
            tbl[p, 2] = float((-1) ** k * comb(k, j)) if valid else 0.0
    return tbl


def _host_coefs(a0, a1):
    tbl = _coef_table()
    c = np.zeros((NT, 1), dtype=np.float32)
    for p in range(NT):
        e0, e1, b = tbl[p]
        if b != 0.0:
            c[p, 0] = np.float32(b) * np.float32(a0) ** np.float32(e0) * np.float32(a1) ** np.float32(e1)
    return c


_NC_CACHE = {}


def _build_nc(mode):
    import concourse.bass as bass
    import concourse.mybir as mybir

    f32 = mybir.dt.float32
    AF = mybir.ActivationFunctionType
    nc = bass.Bass()
    qc = nc.declare_dram_parameter("qc", [QLEN], f32, isOutput=False)
    ncols = 5 if mode == "dev" else 1
    pp = nc.declare_dram_parameter("pp", [NT, ncols], f32, isOutput=False)
    out = nc.declare_dram_parameter("out", [1, 1], f32, isOutput=True)

    with (
        nc.sbuf_tensor([NT, NPOS], f32) as W,
        nc.sbuf_tensor([NT, ncols], f32) as P,
        nc.sbuf_tensor([NT, 2], f32) as lnA,
        nc.sbuf_tensor([NT, 1], f32) as b1,
        nc.sbuf_tensor([NT, 1], f32) as pw,
        nc.sbuf_tensor([NT, 1], f32) as coef,
        nc.psum_tensor([1, NPOS], f32) as x,
        nc.sbuf_tensor([1, NPOS], f32) as sq,
        nc.sbuf_tensor([1, 1], f32) as warm,
        nc.sbuf_tensor([1, 1], f32) as loss,
        nc.semaphore("dsem") as dsem,
        nc.semaphore("asem") as asem,
        nc.semaphore("psem") as psem,
        nc.semaphore("osem") as osem,
        nc.semaphore("csem") as csem,
        nc.Block() as block,
    ):
        a_done = 5 if mode == "dev" else 1  # asem value when loss is ready

        @block.sync
        def _(sync):
            src = bass.AP(qc[:].tensor, 0, [[844, NK], [420, NK], [1, NPOS]])
            sync.dma_start(W[:], src).then_inc(dsem, 16)
            sync.dma_start(P[:], pp[:]).then_inc(dsem, 16)
            sync.wait_ge(asem, a_done)
            sync.dma_start(out[:], loss[:], single_packet=True).then_inc(osem, 16)
            sync.wait_ge(osem, 16)

        @block.scalar
        def _(scalar):
            # Prefetch the activation table set (~2.7us) in parallel with
            # the input DMAs.  Ln anchors natural_log_exp_and_others, which
            # also contains Exp, Copy and Square; warm's input is the
            # preamble-initialized const-1.0 SBUF, its output is never read.
            one = nc.const_aps.aps[(f32, 1.0)][0:1, 0:1]
            scalar.activation(warm[:], one, AF.Ln if mode == "dev" else AF.Square)
            # preamble const memsets (Pool) -> ACT visibility without the
            # preamble all-engine barrier: Pool's nop retires after the
            # memsets (in-order), so csem>=1 makes the float-bias const-0.0
            # reads below safe.  Satisfied ~2us before it is checked.
            scalar.wait_ge(csem, 1)
            if mode == "dev":
                # coef[p] = b_p * exp(e0_p*ln(a0) + e1_p*ln(a1))
                # ACT is pipelined: same-engine RAW hazards need semaphore
                # self-syncs (asem doubles as the flush marker).
                scalar.wait_ge(dsem, 32)
                scalar.activation(lnA[:], P[:, 0:2], AF.Ln).then_inc(asem, 1)
                scalar.wait_ge(asem, 1)
                scalar.mul(b1[:], lnA[:, 1:2], P[:, 3:4]).then_inc(asem, 1)
                scalar.wait_ge(asem, 2)
                scalar.activation(
                    pw[:], P[:, 2:3], AF.Exp, bias=b1[:], scale=lnA[:, 0:1]
                ).then_inc(asem, 1)
                scalar.wait_ge(asem, 3)
                scalar.mul(coef[:], pw[:], P[:, 4:5]).then_inc(asem, 1)
            # loss_partial = sum_i x[0, i]^2
            scalar.wait_ge(psem, 1)
            scalar.activation(
                sq[:], x[:], AF.Square, accum_out=loss[:]
            ).then_inc(asem, 1)

        @block.gpsimd
        def _(gpsimd):
            gpsimd.nop().then_inc(csem, 1)

        @block.tensor
        def _(tensor):
            tensor.wait_ge(dsem, 32)
            if mode == "dev":
                tensor.wait_ge(asem, 4)
                lhsT = coef[:]
            else:
                lhsT = P[:]
            # x[0, i] = sum_p coef[p] * W[p, i]
            tensor.matmul(x[:], lhsT, W[:], start=True, stop=True).then_inc(psem, 1)

    if STRIP_MODE in ("tail", "all"):
        f = nc.m.functions[0]
        last = f.blocks[-1]
        if all(
            type(i).__name__ in ("InstDrain", "InstEventSemaphore")
            for i in last.instructions
        ):
            # our dataflow is fully semaphore-ordered and the out-DMA is
            # completion-waited via osem; the generic end-of-block engine
            # drains add only span time
            last.instructions = []
    if STRIP_MODE == "all":
        b0 = nc.m.functions[0].blocks[0]
        b0.instructions = [
            i
            for i in b0.instructions
            if type(i).__name__ != "InstDrain" and not i.name.startswith("barrier_")
        ]
    return nc


def _get_nc(mode=None):
    mode = mode or COEF_MODE
    if mode not in _NC_CACHE:
        _NC_CACHE[mode] = _build_nc(mode)
    return _NC_CACHE[mode]


def _make_in_maps(q, alpha0, alpha1, mode=None):
    mode = mode or COEF_MODE
    q = np.asarray(q, dtype=np.float32)
    a0 = np.float32(alpha0)
    a1 = np.float32(alpha1)
    qpad = np.concatenate([np.zeros(PAD, np.float32), q[0, 0, :]])
    if mode == "dev":
        tbl = _coef_table()
        pp = np.concatenate(
            [np.full((NT, 1), a0, np.float32), np.full((NT, 1), a1, np.float32), tbl],
            axis=1,
        )
    else:
        pp = _host_coefs(a0, a1)
    return [
        {
            "qc": np.ascontiguousarray(qpad[NPOS * c : NPOS * c + QLEN]),
            "pp": pp,
        }
        for c in range(NCORES)
    ]


def _run(q, alpha0, alpha1, mode=None, **spmd_kwargs):
    from concourse.bass_utils import run_bass_kernel_spmd

    mode = mode or COEF_MODE
    in_maps = _make_in_maps(q, alpha0, alpha1, mode)
    nc = _get_nc(mode)
    res = run_bass_kernel_spmd(nc, in_maps, core_ids=list(range(NCORES)), **spmd_kwargs)
    total = np.float32(0.0)
    for r in res.results:
        total = np.float32(total + r["out"][0, 0])
    return np.array(total, dtype=np.float32), res


def kernel(q, alpha0, alpha1):
    out, _ = _run(q, alpha0, alpha1)
    return out
